# revision 1
# baseline (speedup 1.0000x reference)
"""Trainium2 Bass kernel for paged-attention Llama-style block (nn_L4maAttention).

Sharding: tensor-parallel over heads across 8 NeuronCores. Core c owns
q-heads [4c, 4c+4), kv-head c, wq/wk/wv row shards and the matching wo
column shard. Each core computes a full [T, HID] partial of the output
projection; the host sums the 8 partials (the TP reduce) after gathering.

Device kernel (per core), matmuls in fp32r (full PE rate at free dim 512):
  phase 1: QT/KT/VT projections from hT (hidden states transposed on host)
           + Llama-3.1 RoPE on Q/K + PE-transpose of VT -> V (token-major).
  phase 2: causal attention per (batch, head) with transposed scores
           [k-tokens on partitions, q-tokens free]; softmax without
           max-subtraction (scaled scores are small); denominators via a
           ones-matmul (naturally broadcast over partitions); normalization
           applied to the PV output.
  phase 3: output projection partial = attn_outT.T @ woT.
"""

import math
import sys
from contextlib import ExitStack

import numpy as np

for _p in ("/opt/trn_rl_repo",):
    if _p not in sys.path:
        sys.path.insert(0, _p)

import concourse.mybir as mybir  # noqa: E402
import concourse.tile as tile  # noqa: E402
from concourse import bacc  # noqa: E402
from concourse.bass_utils import run_bass_kernel_spmd  # noqa: E402

NCORES = 8
HID = 4096
NH = 32
NKV = 8
HD = 128
B = 4
S = 1024
T = B * S
GQ = NH // NCORES          # q heads per core = 4
DQ = GQ * HD               # 512
TCH = 512                  # token chunk (matmul moving free dim)
NTCH = T // TCH            # 8
KT32 = HID // 128          # 32 k tiles
SCALE = 1.0 / math.sqrt(HD)

FP32 = mybir.dt.float32
FP32R = mybir.dt.float32r

_PROG_CACHE: dict = {}


def _llama31_freqs_np(head_dim: int) -> np.ndarray:
    half = head_dim // 2
    theta, scale, low_ff, high_ff, old_ctx = 500000.0, 8.0, 1.0, 4.0, 8192.0
    freq = 1.0 / (theta ** (np.arange(half, dtype=np.float64) * 2.0 / head_dim))
    wavelen = 2.0 * np.pi / freq
    low_wl, high_wl = old_ctx / low_ff, old_ctx / high_ff
    smooth = (old_ctx / wavelen - low_ff) / (high_ff - low_ff)
    out = np.where(
        wavelen < high_wl,
        freq,
        np.where(wavelen > low_wl, freq / scale, (1.0 - smooth) * freq / scale + smooth * freq),
    )
    return out.astype(np.float64)


def _rope_tables(pos: np.ndarray) -> tuple[np.ndarray, np.ndarray]:
    """cosF [128, n]: cos duplicated on both partition halves.
    sinF2 [128, n]: +sin on rows 0-63, -sin on rows 64-127. The kernel
    computes out = x*cosF + halfswap(x*sinF2), which equals rotate-half RoPE."""
    freqs = _llama31_freqs_np(HD)
    ang = pos.astype(np.float64)[None, :] * freqs[:, None]  # [64, n]
    c = np.cos(ang).astype(np.float32)
    s = np.sin(ang).astype(np.float32)
    cosF = np.concatenate([c, c], axis=0)
    sinF2 = np.concatenate([s, -s], axis=0)
    return np.ascontiguousarray(cosF), np.ascontiguousarray(sinF2)


def _masks_np() -> np.ndarray:
    """4 diagonal-block masks [128, 4*512]: mask[r][j, i] = (128*r + j) <= i."""
    m = np.zeros((128, 4 * TCH), np.float32)
    j = np.arange(128)[:, None]
    i = np.arange(TCH)[None, :]
    for r in range(4):
        m[:, r * TCH:(r + 1) * TCH] = ((128 * r + j) <= i).astype(np.float32)
    return m


def _build_program(split_kv: bool):
    nc = bacc.Bacc(
        "TRN2",
        target_bir_lowering=False,
        debug=False,
        enable_asserts=False,
        num_devices=NCORES,
    )
    hT = nc.dram_tensor("hT", [HID, T], FP32R, kind="ExternalInput")
    hTkv = (
        nc.dram_tensor("hTkv", [HID, T], FP32R, kind="ExternalInput") if split_kv else hT
    )
    wqT = nc.dram_tensor("wqT", [HID, DQ], FP32R, kind="ExternalInput")
    wkT = nc.dram_tensor("wkT", [HID, HD], FP32R, kind="ExternalInput")
    wvT = nc.dram_tensor("wvT", [HID, HD], FP32R, kind="ExternalInput")
    woT = nc.dram_tensor("woT", [DQ, HID], FP32R, kind="ExternalInput")
    cosq = nc.dram_tensor("cosq", [128, T], FP32, kind="ExternalInput")
    sinq = nc.dram_tensor("sinq", [128, T], FP32, kind="ExternalInput")
    if split_kv:
        cosk = nc.dram_tensor("cosk", [128, T], FP32, kind="ExternalInput")
        sink = nc.dram_tensor("sink", [128, T], FP32, kind="ExternalInput")
    else:
        cosk, sink = cosq, sinq
    maskd = nc.dram_tensor("maskd", [128, 4 * TCH], FP32, kind="ExternalInput")
    identd = nc.dram_tensor("identd", [128, 128], FP32R, kind="ExternalInput")
    onesd = nc.dram_tensor("onesd", [128, 128], FP32R, kind="ExternalInput")
    outp = nc.dram_tensor("outp", [T, HID], FP32, kind="ExternalOutput")

    with tile.TileContext(nc) as tc, ExitStack() as ctx:
        const_pool = ctx.enter_context(tc.tile_pool(name="const", bufs=1))
        dram_pool = ctx.enter_context(tc.tile_pool(name="dram", bufs=1, space="DRAM"))
        res_pool = ctx.enter_context(tc.tile_pool(name="res", bufs=1))

        ident = const_pool.tile([128, 128], FP32R)
        nc.sync.dma_start(ident[:], identd.ap()[:, :])
        ones_sb = const_pool.tile([128, 128], FP32R)
        nc.sync.dma_start(ones_sb[:], onesd.ap()[:, :])
        KT_res = res_pool.tile([128, T], FP32R)            # 16KB/part
        V_res = res_pool.tile([128, T // 128, HD], FP32R)  # 16KB/part
        QT_dram = dram_pool.tile([GQ, 128, T], FP32R)

        # ---------------- phase 1: projections + rope + V transpose ---------
        with tc.tile_pool(name="w1", bufs=1) as wpool, \
             tc.tile_pool(name="h1", bufs=4) as hpool, \
             tc.tile_pool(name="cs1", bufs=2) as cspool, \
             tc.tile_pool(name="ps1", bufs=1, space="PSUM") as ppool, \
             tc.tile_pool(name="pst1", bufs=2, space="PSUM") as ptpool, \
             tc.tile_pool(name="st1", bufs=2) as stpool:
            wq_sb = wpool.tile([128, KT32, DQ], FP32R)
            wk_sb = wpool.tile([128, KT32, HD], FP32R)
            wv_sb = wpool.tile([128, KT32, HD], FP32R)
            wq_r = wqT.ap().rearrange("(ko p) d -> p ko d", p=128)
            wk_r = wkT.ap().rearrange("(ko p) d -> p ko d", p=128)
            wv_r = wvT.ap().rearrange("(ko p) d -> p ko d", p=128)
            for kg in range(0, KT32, 4):
                nc.sync.dma_start(wq_sb[:, kg:kg + 4, :], wq_r[:, kg:kg + 4, :])
                nc.sync.dma_start(wk_sb[:, kg:kg + 4, :], wk_r[:, kg:kg + 4, :])
                nc.sync.dma_start(wv_sb[:, kg:kg + 4, :], wv_r[:, kg:kg + 4, :])

            for tci in range(NTCH):
                tsl = slice(tci * TCH, (tci + 1) * TCH)
                cos_t = cspool.tile([128, TCH], FP32, tag="cosq")
                sin_t = cspool.tile([128, TCH], FP32, tag="sinq")
                nc.sync.dma_start(cos_t[:], cosq.ap()[:, tsl])
                nc.sync.dma_start(sin_t[:], sinq.ap()[:, tsl])
                if split_kv:
                    cosk_t = cspool.tile([128, TCH], FP32, tag="cosk")
                    sink_t = cspool.tile([128, TCH], FP32, tag="sink")
                    nc.sync.dma_start(cosk_t[:], cosk.ap()[:, tsl])
                    nc.sync.dma_start(sink_t[:], sink.ap()[:, tsl])
                else:
                    cosk_t, sink_t = cos_t, sin_t

                psq = [
                    ppool.tile([128, TCH], FP32, tag=f"psq{g}", name=f"psq{g}")
                    for g in range(GQ)
                ]
                psk = ppool.tile([128, TCH], FP32, tag="psk")
                psv = ppool.tile([128, TCH], FP32, tag="psv")
                for k in range(KT32):
                    h_t = hpool.tile([128, TCH], FP32R, tag="h")
                    nc.sync.dma_start(h_t[:], hT.ap()[k * 128:(k + 1) * 128, tsl])
                    if split_kv:
                        hkv_t = hpool.tile([128, TCH], FP32R, tag="hkv")
                        nc.sync.dma_start(
                            hkv_t[:], hTkv.ap()[k * 128:(k + 1) * 128, tsl]
                        )
                    else:
                        hkv_t = h_t
                    st = k == 0
                    sp = k == KT32 - 1
                    for g in range(GQ):
                        nc.tensor.matmul(
                            psq[g][:],
                            wq_sb[:, k, g * 128:(g + 1) * 128],
                            h_t[:], start=st, stop=sp,
                        )
                    nc.tensor.matmul(
                        psk[:], wk_sb[:, k, :], hkv_t[:], start=st, stop=sp
                    )
                    nc.tensor.matmul(
                        psv[:], wv_sb[:, k, :], hkv_t[:], start=st, stop=sp
                    )

                # RoPE: out = x*cosF + halfswap(x*sinF2)
                def _rope(ps, cos_a, sin_a, out_ap):
                    t1 = stpool.tile([128, TCH], FP32, tag="t1")
                    nc.vector.tensor_mul(t1[:], ps[:], cos_a[:])
                    t2r = stpool.tile([128, TCH], FP32, tag="t2r")
                    nc.vector.tensor_mul(t2r[:], ps[:], sin_a[:])
                    t2 = stpool.tile([128, TCH], FP32, tag="t2")
                    nc.sync.dma_start(t2[0:64, :], t2r[64:128, :])
                    nc.sync.dma_start(t2[64:128, :], t2r[0:64, :])
                    nc.vector.tensor_add(out_ap, t1[:], t2[:])

                for g in range(GQ):
                    qo = stpool.tile([128, TCH], FP32R, tag="qo")
                    _rope(psq[g], cos_t, sin_t, qo[:])
                    nc.sync.dma_start(QT_dram[g, :, tsl], qo[:])
                _rope(psk, cosk_t, sink_t, KT_res[:, tsl])

                vsb = stpool.tile([128, TCH], FP32R, tag="vsb")
                nc.scalar.copy(vsb[:], psv[:])
                for tb in range(4):
                    pst = ptpool.tile([128, 128], FP32R, tag="pst")
                    nc.tensor.transpose(pst[:], vsb[:, tb * 128:(tb + 1) * 128], ident[:])
                    nc.vector.tensor_copy(V_res[:, tci * 4 + tb, :], pst[:])

        # ---------------- phase 2: attention ---------------------------------
        res23_pool = ctx.enter_context(tc.tile_pool(name="res23", bufs=1))
        aoT_res = res23_pool.tile([128, GQ, T], FP32R)  # 64KB/part (after w1 freed)
        with tc.tile_pool(name="qt2", bufs=2) as qtpool, \
             tc.tile_pool(name="ex2", bufs=4) as expool, \
             tc.tile_pool(name="msk2", bufs=1) as mskpool, \
             tc.tile_pool(name="ps2", bufs=2, space="PSUM") as p2pool:
            mask_sb = mskpool.tile([128, 4 * TCH], FP32)
            nc.sync.dma_start(mask_sb[:], maskd.ap()[:, :])
            for b in range(B):
                for g in range(GQ):
                    qt = qtpool.tile([128, S], FP32R, tag="qt")
                    nc.sync.dma_start(qt[:], QT_dram[g, :, b * S:(b + 1) * S])
                    for it in range(2):
                        njt = 4 * (it + 1)
                        es = expool.tile([128, TCH], FP32R, tag="es")
                        pv = p2pool.tile([128, TCH], FP32, tag="pv")
                        for jt in range(njt):
                            pss = p2pool.tile([128, TCH], FP32, tag="pss")
                            nc.tensor.matmul(
                                pss[:],
                                KT_res[:, b * S + jt * 128: b * S + (jt + 1) * 128],
                                qt[:, it * TCH:(it + 1) * TCH],
                                start=True, stop=True,
                            )
                            ex = expool.tile([128, TCH], FP32R, tag="ex")
                            nc.scalar.activation(
                                ex[:], pss[:], mybir.ActivationFunctionType.Exp,
                                scale=SCALE,
                            )
                            r = jt - 4 * it
                            if r >= 0:  # diagonal block -> causal mask
                                nc.vector.tensor_mul(
                                    ex[:], ex[:], mask_sb[:, r * TCH:(r + 1) * TCH]
                                )
                            if jt == 0:
                                nc.vector.tensor_copy(es[:], ex[:])
                            else:
                                nc.vector.tensor_add(es[:], es[:], ex[:])
                            nc.tensor.matmul(
                                pv[:],
                                V_res[:, b * 8 + jt, :],
                                ex[:],
                                start=(jt == 0), stop=(jt == njt - 1),
                            )
                        psd = p2pool.tile([128, TCH], FP32, tag="psd")
                        nc.tensor.matmul(
                            psd[:], ones_sb[:], es[:], start=True, stop=True
                        )
                        rec = expool.tile([128, TCH], FP32, tag="rec")
                        nc.vector.reciprocal(rec[:], psd[:])
                        nc.vector.tensor_mul(
                            aoT_res[:, g, b * S + it * TCH: b * S + (it + 1) * TCH],
                            pv[:], rec[:],
                        )

        # ---------------- phase 3: output projection partial ------------------
        with tc.tile_pool(name="wo3", bufs=8) as wopool, \
             tc.tile_pool(name="ps3", bufs=8, space="PSUM") as p3pool, \
             tc.tile_pool(name="ob3", bufs=6) as obpool:
            wo_r = woT.ap().rearrange("(g p) e -> p g e", p=128)
            for e in range(8):
                esl = slice(e * TCH, (e + 1) * TCH)
                wo_t = []
                for g in range(GQ):
                    w = wopool.tile([128, TCH], FP32R, tag="wo")
                    nc.sync.dma_start(w[:], wo_r[:, g, esl])
                    wo_t.append(w)
                for tb in range(T // 128):
                    pso = p3pool.tile([128, TCH], FP32, tag="pso")
                    for g in range(GQ):
                        nc.tensor.matmul(
                            pso[:],
                            aoT_res[:, g, tb * 128:(tb + 1) * 128],
                            wo_t[g][:],
                            start=(g == 0), stop=(g == GQ - 1),
                        )
                    ob = obpool.tile([128, TCH], FP32)
                    nc.scalar.copy(ob[:], pso[:])
                    nc.sync.dma_start(outp.ap()[tb * 128:(tb + 1) * 128, esl], ob[:])

    nc.finalize()
    return nc


def _get_program(split_kv: bool):
    if split_kv not in _PROG_CACHE:
        _PROG_CACHE[split_kv] = _build_program(split_kv)
    return _PROG_CACHE[split_kv]


def kernel(
    hidden_states, wq, wk, wv, wo, kv_cache, position_ids,
    kv_page_indices, kv_page_indptr, kv_last_page_lens, qo_indptr,
    _run_kwargs: dict | None = None,
):
    hidden_states = np.asarray(hidden_states, np.float32)
    wq = np.asarray(wq, np.float32)
    wk = np.asarray(wk, np.float32)
    wv = np.asarray(wv, np.float32)
    wo = np.asarray(wo, np.float32)
    position_ids = np.asarray(position_ids, np.int32)
    qo_indptr = np.asarray(qo_indptr, np.int64)

    nnz = hidden_states.shape[0]
    b = qo_indptr.shape[0] - 1
    assert nnz == T and b == B, (nnz, b)
    assert np.array_equal(qo_indptr, np.arange(B + 1, dtype=np.int64) * S), (
        "kernel assumes uniform sequence lengths of 1024"
    )

    # Page-gather order: the reference gathers pages in list order, so the
    # token with position p within its sequence lands at page-order rank p.
    # KV must be fed in rank order; the q path stays in token order.
    perm = np.empty(T, np.int64)
    identity = True
    for bi in range(B):
        pos_b = position_ids[bi * S:(bi + 1) * S].astype(np.int64)
        assert np.array_equal(np.sort(pos_b), np.arange(S)), (
            "kernel assumes positions cover 0..S-1 exactly once per sequence"
        )
        inv = np.empty(S, np.int64)
        inv[pos_b] = np.arange(S)
        perm[bi * S:(bi + 1) * S] = bi * S + inv
        if not np.array_equal(inv, np.arange(S)):
            identity = False

    hT = np.ascontiguousarray(hidden_states.T)
    cosq, sinq = _rope_tables(position_ids)
    maskd = _masks_np()
    eye = np.eye(128, dtype=np.float32)
    ones = np.ones((128, 128), np.float32)

    split_kv = not identity
    nc = _get_program(split_kv)

    in_maps = []
    for c in range(NCORES):
        im = {
            "hT": hT,
            "wqT": np.ascontiguousarray(wq[c * DQ:(c + 1) * DQ, :].T),
            "wkT": np.ascontiguousarray(wk[c * HD:(c + 1) * HD, :].T),
            "wvT": np.ascontiguousarray(wv[c * HD:(c + 1) * HD, :].T),
            "woT": np.ascontiguousarray(wo[:, c * DQ:(c + 1) * DQ].T),
            "cosq": cosq,
            "sinq": sinq,
            "maskd": maskd,
            "identd": eye,
            "onesd": ones,
        }
        if split_kv:
            im["hTkv"] = np.ascontiguousarray(hT[:, perm])
            cosk, sink = _rope_tables(position_ids[perm])
            im["cosk"] = cosk
            im["sink"] = sink
        in_maps.append(im)

    res = run_bass_kernel_spmd(
        nc, in_maps, core_ids=list(range(NCORES)), **(_run_kwargs or {})
    )
    out = np.zeros((T, HID), np.float64)
    for c in range(NCORES):
        out += res.results[c]["outp"].astype(np.float64)
    kernel.last_results = res  # type: ignore[attr-defined]
    return out.astype(np.float32)



# revision 5
# speedup vs baseline: 1.4037x; 1.4037x over previous
"""Trainium2 Bass kernel for paged-attention Llama-style block (nn_L4maAttention).

Sharding: tensor-parallel over heads across 8 NeuronCores. Core c owns
q-heads [4c, 4c+4), kv-head c, wq/wk/wv row shards and the matching wo
column shard. Each core computes a full [T, HID] partial of the output
projection in bf16; the host sums the 8 partials (the TP reduce).

Device kernel (per core), matmuls in bf16 (full PE rate, half DMA):
  phase 1a: K/V projections in 16 token chunks of 256. K^T computed
            head-dim-major + RoPE -> KT (SBUF resident). V computed
            token-major directly (h tile as the stationary operand), no
            transposes. PSUM double-buffered (3 banks x 2) so chunk
            evacuation overlaps the next chunk's matmuls.
  phase 1b: Q projections in 8 chunks of 512, 4 heads x [128,512] PSUM
            (4 banks x 2 = all 8), RoPE -> QT (SBUF resident). RoPE
            half-swap done with partition-offset vector adds (no DMA).
  phase 2:  causal attention per (batch, head, q-half): transposed
            scores [k on partitions, q free] computed only on the
            causal suffix of each 128-row k block; exp (bf16) with the
            1/sqrt(d) folded into the activation scale; single [128,128]
            triangular mask on the diagonal sub-block; denominators via
            a ones-matmul (broadcasts over partitions) + fast reciprocal.
  phase 3:  output projection partial = aoT.T @ woT in bf16, PSUM
            evacuation alternating scalar/vector engines, 512KB output
            DMAs.
"""

import math
import sys
from contextlib import ExitStack

import numpy as np

for _p in ("/opt/trn_rl_repo",):
    if _p not in sys.path:
        sys.path.insert(0, _p)

import concourse.mybir as mybir  # noqa: E402
import concourse.tile as tile  # noqa: E402
from concourse import bacc  # noqa: E402
from concourse.bass_utils import run_bass_kernel_spmd  # noqa: E402

NCORES = 8
HID = 4096
NH = 32
NKV = 8
HD = 128
B = 4
S = 1024
T = B * S
GQ = NH // NCORES          # q heads per core = 4
DQ = GQ * HD               # 512
KT32 = HID // 128          # 32 k tiles
KCH = 256                  # token chunk, K/V pass
NKCH = T // KCH            # 16
QCH = 512                  # token chunk, Q pass
NQCH = T // QCH            # 8
SCALE = 1.0 / math.sqrt(HD)

FP32 = mybir.dt.float32
BF16 = mybir.dt.bfloat16
NP_BF16 = mybir.dt.np(BF16)

_PROG_CACHE: dict = {}


def _llama31_freqs_np(head_dim: int) -> np.ndarray:
    half = head_dim // 2
    theta, scale, low_ff, high_ff, old_ctx = 500000.0, 8.0, 1.0, 4.0, 8192.0
    freq = 1.0 / (theta ** (np.arange(half, dtype=np.float64) * 2.0 / head_dim))
    wavelen = 2.0 * np.pi / freq
    low_wl, high_wl = old_ctx / low_ff, old_ctx / high_ff
    smooth = (old_ctx / wavelen - low_ff) / (high_ff - low_ff)
    out = np.where(
        wavelen < high_wl,
        freq,
        np.where(wavelen > low_wl, freq / scale, (1.0 - smooth) * freq / scale + smooth * freq),
    )
    return out.astype(np.float64)


def _rope_tables(pos: np.ndarray) -> tuple[np.ndarray, np.ndarray]:
    """cosF [128, n]: cos duplicated on both partition halves.
    sinF2 [128, n]: +sin on rows 0-63, -sin on rows 64-127. The kernel
    computes out = x*cosF + halfswap(x*sinF2), which equals rotate-half
    RoPE (the halfswap is done with partition-offset adds)."""
    freqs = _llama31_freqs_np(HD)
    ang = pos.astype(np.float64)[None, :] * freqs[:, None]  # [64, n]
    c = np.cos(ang).astype(np.float32)
    s = np.sin(ang).astype(np.float32)
    cosF = np.concatenate([c, c], axis=0)
    sinF2 = np.concatenate([s, -s], axis=0)
    return np.ascontiguousarray(cosF), np.ascontiguousarray(sinF2)


def _build_program(split_kv: bool):
    nc = bacc.Bacc(
        "TRN2",
        target_bir_lowering=False,
        debug=False,
        enable_asserts=False,
        num_devices=NCORES,
    )
    hT = nc.dram_tensor("hT", [HID, T], BF16, kind="ExternalInput")
    hTkv = (
        nc.dram_tensor("hTkv", [HID, T], BF16, kind="ExternalInput") if split_kv else hT
    )
    wqT = nc.dram_tensor("wqT", [HID, DQ], BF16, kind="ExternalInput")
    wkT = nc.dram_tensor("wkT", [HID, HD], BF16, kind="ExternalInput")
    wvT = nc.dram_tensor("wvT", [HID, HD], BF16, kind="ExternalInput")
    woT = nc.dram_tensor("woT", [DQ, HID], BF16, kind="ExternalInput")
    # K (page-rank order) positions are always 0..S-1 per sequence; a
    # [128, S] table sliced modulo S covers both passes in the identity
    # case and the KV pass in the permuted case.
    coskv = nc.dram_tensor("coskv", [128, S], FP32, kind="ExternalInput")
    sinkv = nc.dram_tensor("sinkv", [128, S], FP32, kind="ExternalInput")
    if split_kv:
        cosq = nc.dram_tensor("cosq", [128, T], FP32, kind="ExternalInput")
        sinq = nc.dram_tensor("sinq", [128, T], FP32, kind="ExternalInput")
    trid = nc.dram_tensor("trid", [128, 128], BF16, kind="ExternalInput")
    onesd = nc.dram_tensor("onesd", [128, 128], BF16, kind="ExternalInput")
    outp = nc.dram_tensor("outp", [T, HID], BF16, kind="ExternalOutput")

    with tile.TileContext(nc) as tc, ExitStack() as ctx:
        const_pool = ctx.enter_context(tc.tile_pool(name="const", bufs=1))
        QT = const_pool.tile([128, GQ, T], BF16)        # 32KB/part
        KT = const_pool.tile([128, T], BF16)            # 8KB
        V = const_pool.tile([128, T // 128, HD], BF16)  # 8KB (token-major tiles)
        aoT = const_pool.tile([128, GQ, T], BF16)       # 32KB
        tri_sb = const_pool.tile([128, 128], BF16)
        ones_sb = const_pool.tile([128, 128], BF16)
        coskv_sb = const_pool.tile([128, S], FP32)
        sinkv_sb = const_pool.tile([128, S], FP32)
        nc.sync.dma_start(tri_sb[:], trid.ap()[:, :])
        nc.sync.dma_start(ones_sb[:], onesd.ap()[:, :])
        nc.sync.dma_start(coskv_sb[:], coskv.ap()[:, :])
        nc.sync.dma_start(sinkv_sb[:], sinkv.ap()[:, :])
        if split_kv:
            cosq_sb = const_pool.tile([128, T], FP32)
            sinq_sb = const_pool.tile([128, T], FP32)
            nc.sync.dma_start(cosq_sb[:], cosq.ap()[:, :])
            nc.sync.dma_start(sinq_sb[:], sinq.ap()[:, :])
        else:
            cosq_sb, sinq_sb = coskv_sb, sinkv_sb

        def rope_out(ps, cos_a, sin_a, out_full, tpool, n):
            """out = ps*cos + halfswap(ps*sinF2). The half-swap is two
            partition-shifted SBUF->SBUF DMAs (TensorTensor ops must have
            all operands on the same start partition)."""
            t1 = tpool.tile([128, n], FP32, tag="t1", name="t1")
            u = tpool.tile([128, n], FP32, tag="u", name="u")
            u2 = tpool.tile([128, n], FP32, tag="u2", name="u2")
            nc.vector.tensor_mul(t1[:], ps[:], cos_a)
            nc.vector.tensor_mul(u[:], ps[:], sin_a)
            nc.sync.dma_start(u2[0:64, :], u[64:128, :])
            nc.sync.dma_start(u2[64:128, :], u[0:64, :])
            nc.vector.tensor_add(out_full, t1[:], u2[:])

        # ---------------- phase 1a: K/V projections ---------------------
        hkv_r = hTkv.ap().rearrange("(ko p) t -> p ko t", p=128)
        wk_r = wkT.ap().rearrange("(ko p) d -> p ko d", p=128)
        wv_r = wvT.ap().rearrange("(ko p) d -> p ko d", p=128)
        with tc.tile_pool(name="wkv", bufs=1) as wkvpool, \
             tc.tile_pool(name="h1a", bufs=6) as hpool, \
             tc.tile_pool(name="ps1a", bufs=2, space="PSUM") as ppool, \
             tc.tile_pool(name="st1a", bufs=2) as stpool:
            wk_sb = wkvpool.tile([128, KT32, HD], BF16)
            wv_sb = wkvpool.tile([128, KT32, HD], BF16)
            for kg in range(0, KT32, 8):
                nc.sync.dma_start(wk_sb[:, kg:kg + 8, :], wk_r[:, kg:kg + 8, :])
                nc.sync.dma_start(wv_sb[:, kg:kg + 8, :], wv_r[:, kg:kg + 8, :])
            for c in range(NKCH):
                tsl = slice(c * KCH, (c + 1) * KCH)
                hts = []
                for j in range(4):
                    ht = hpool.tile([128, 8, KCH], BF16, tag="h", name="h")
                    nc.sync.dma_start(ht[:], hkv_r[:, j * 8:(j + 1) * 8, tsl])
                    hts.append(ht)
                psk = ppool.tile([128, KCH], FP32, tag="psk", name="psk")
                psv0 = ppool.tile([128, HD], FP32, tag="psv0", name="psv0")
                psv1 = ppool.tile([128, HD], FP32, tag="psv1", name="psv1")
                for k in range(KT32):
                    ht = hts[k // 8][:, k % 8, :]
                    st = k == 0
                    sp = k == KT32 - 1
                    nc.tensor.matmul(psk[:], wk_sb[:, k, :], ht, start=st, stop=sp)
                    nc.tensor.matmul(
                        psv0[:], hts[k // 8][:, k % 8, 0:128], wv_sb[:, k, :],
                        start=st, stop=sp,
                    )
                    nc.tensor.matmul(
                        psv1[:], hts[k // 8][:, k % 8, 128:256], wv_sb[:, k, :],
                        start=st, stop=sp,
                    )
                p0 = (c * KCH) % S
                rope_out(
                    psk, coskv_sb[:, p0:p0 + KCH], sinkv_sb[:, p0:p0 + KCH],
                    KT[:, tsl], stpool, KCH,
                )
                nc.scalar.copy(V[:, 2 * c, :], psv0[:])
                nc.scalar.copy(V[:, 2 * c + 1, :], psv1[:])

        # ---------------- phase 1b: Q projections ------------------------
        h_r = hT.ap().rearrange("(ko p) t -> p ko t", p=128)
        wq_r = wqT.ap().rearrange("(ko p) d -> p ko d", p=128)
        with tc.tile_pool(name="wq", bufs=1) as wqpool, \
             tc.tile_pool(name="h1b", bufs=6) as hpool, \
             tc.tile_pool(name="ps1b", bufs=2, space="PSUM") as ppool, \
             tc.tile_pool(name="st1b", bufs=2) as stpool:
            wq_sb = wqpool.tile([128, KT32, DQ], BF16)
            for kg in range(0, KT32, 4):
                nc.sync.dma_start(wq_sb[:, kg:kg + 4, :], wq_r[:, kg:kg + 4, :])
            for c in range(NQCH):
                tsl = slice(c * QCH, (c + 1) * QCH)
                hts = []
                for j in range(4):
                    ht = hpool.tile([128, 8, QCH], BF16, tag="h", name="h")
                    nc.sync.dma_start(ht[:], h_r[:, j * 8:(j + 1) * 8, tsl])
                    hts.append(ht)
                psq = [
                    ppool.tile([128, QCH], FP32, tag=f"psq{g}", name=f"psq{g}")
                    for g in range(GQ)
                ]
                for k in range(KT32):
                    ht = hts[k // 8][:, k % 8, :]
                    st = k == 0
                    sp = k == KT32 - 1
                    for g in range(GQ):
                        nc.tensor.matmul(
                            psq[g][:], wq_sb[:, k, g * 128:(g + 1) * 128], ht,
                            start=st, stop=sp,
                        )
                if split_kv:
                    cs, ss = cosq_sb[:, tsl], sinq_sb[:, tsl]
                else:
                    p0 = (c * QCH) % S
                    cs, ss = cosq_sb[:, p0:p0 + QCH], sinq_sb[:, p0:p0 + QCH]
                for g in range(GQ):
                    rope_out(psq[g], cs, ss, QT[:, g, tsl], stpool, QCH)

        # ---------------- phase 2: attention ------------------------------
        wo_r = woT.ap().rearrange("(g p) e -> p g e", p=128)
        with tc.tile_pool(name="wo", bufs=1) as wopool:
            wo_sb = wopool.tile([128, GQ, HID], BF16)
            for g in range(GQ):
                nc.sync.dma_start(wo_sb[:, g, :], wo_r[:, g, :])

            with tc.tile_pool(name="sb2", bufs=2) as sbpool, \
                 tc.tile_pool(name="pss2", bufs=2, space="PSUM") as pspool, \
                 tc.tile_pool(name="pv2", bufs=2, space="PSUM") as pvpool, \
                 tc.tile_pool(name="psd2", bufs=2, space="PSUM") as pdpool:
                for b in range(B):
                    for g in range(GQ):
                        for it in range(2):
                            qoff = it * QCH
                            q0 = b * S + qoff
                            njt = (qoff + QCH) // 128
                            es = sbpool.tile([128, QCH], BF16, tag="es", name="es")
                            pv = pvpool.tile([128, QCH], FP32, tag="pv", name="pv")
                            for jt in range(njt):
                                ko = b * S + jt * 128
                                off = jt * 128 - qoff if jt * 128 >= qoff else 0
                                pss = pspool.tile(
                                    [128, QCH], FP32, tag="pss", name="pss"
                                )
                                nc.tensor.matmul(
                                    pss[:, off:QCH],
                                    KT[:, ko:ko + 128],
                                    QT[:, g, q0 + off:q0 + QCH],
                                    start=True, stop=True,
                                )
                                ex = sbpool.tile([128, QCH], BF16, tag="ex", name="ex")
                                nc.scalar.activation(
                                    ex[:, off:QCH], pss[:, off:QCH],
                                    mybir.ActivationFunctionType.Exp, scale=SCALE,
                                )
                                if jt * 128 >= qoff:  # diagonal block
                                    nc.vector.tensor_mul(
                                        ex[:, off:off + 128], ex[:, off:off + 128],
                                        tri_sb[:],
                                    )
                                if jt == 0:
                                    nc.vector.tensor_copy(es[:], ex[:])
                                else:
                                    nc.vector.tensor_add(
                                        es[:, off:QCH], es[:, off:QCH], ex[:, off:QCH]
                                    )
                                nc.tensor.matmul(
                                    pv[:, off:QCH],
                                    V[:, b * 8 + jt, :],
                                    ex[:, off:QCH],
                                    start=(jt == 0), stop=(jt == njt - 1),
                                )
                            psd = pdpool.tile([128, QCH], FP32, tag="psd", name="psd")
                            nc.tensor.matmul(
                                psd[:], ones_sb[:], es[:], start=True, stop=True
                            )
                            rec = sbpool.tile([128, QCH], FP32, tag="rec", name="rec")
                            nc.vector.reciprocal_approx_fast(rec[:], psd[:])
                            nc.vector.tensor_mul(
                                aoT[:, g, q0:q0 + QCH], pv[:], rec[:]
                            )

            # ---------------- phase 3: output projection partial ----------
            with tc.tile_pool(name="ps3", bufs=6, space="PSUM") as p3pool, \
                 tc.tile_pool(name="ob3", bufs=3) as obpool:
                for eg in range(2):
                    for tb in range(T // 128):
                        ob = obpool.tile([128, 4, QCH], BF16, tag="ob", name="ob")
                        for ei in range(4):
                            e0 = eg * 2048 + ei * QCH
                            pso = p3pool.tile([128, QCH], FP32, tag="pso", name="pso")
                            for g in range(GQ):
                                nc.tensor.matmul(
                                    pso[:],
                                    aoT[:, g, tb * 128:(tb + 1) * 128],
                                    wo_sb[:, g, e0:e0 + QCH],
                                    start=(g == 0), stop=(g == GQ - 1),
                                )
                            if ei % 2 == 0:
                                nc.scalar.copy(ob[:, ei, :], pso[:])
                            else:
                                nc.vector.tensor_copy(ob[:, ei, :], pso[:])
                        nc.sync.dma_start(
                            outp.ap()[tb * 128:(tb + 1) * 128,
                                      eg * 2048:(eg + 1) * 2048],
                            ob[:],
                        )

    nc.finalize()
    return nc


def _get_program(split_kv: bool):
    if split_kv not in _PROG_CACHE:
        _PROG_CACHE[split_kv] = _build_program(split_kv)
    return _PROG_CACHE[split_kv]


def kernel(
    hidden_states, wq, wk, wv, wo, kv_cache, position_ids,
    kv_page_indices, kv_page_indptr, kv_last_page_lens, qo_indptr,
    _run_kwargs: dict | None = None,
):
    hidden_states = np.asarray(hidden_states, np.float32)
    wq = np.asarray(wq, np.float32)
    wk = np.asarray(wk, np.float32)
    wv = np.asarray(wv, np.float32)
    wo = np.asarray(wo, np.float32)
    position_ids = np.asarray(position_ids, np.int32)
    qo_indptr = np.asarray(qo_indptr, np.int64)

    nnz = hidden_states.shape[0]
    b = qo_indptr.shape[0] - 1
    assert nnz == T and b == B, (nnz, b)
    assert np.array_equal(qo_indptr, np.arange(B + 1, dtype=np.int64) * S), (
        "kernel assumes uniform sequence lengths of 1024"
    )

    # Page-gather order: the reference gathers pages in list order, so the
    # token with position p within its sequence lands at page-order rank p.
    # KV must be fed in rank order; the q path stays in token order.
    perm = np.empty(T, np.int64)
    identity = True
    for bi in range(B):
        pos_b = position_ids[bi * S:(bi + 1) * S].astype(np.int64)
        assert np.array_equal(np.sort(pos_b), np.arange(S)), (
            "kernel assumes positions cover 0..S-1 exactly once per sequence"
        )
        inv = np.empty(S, np.int64)
        inv[pos_b] = np.arange(S)
        perm[bi * S:(bi + 1) * S] = bi * S + inv
        if not np.array_equal(inv, np.arange(S)):
            identity = False

    hT16 = np.ascontiguousarray(hidden_states.T.astype(NP_BF16))
    coskv, sinkv = _rope_tables(np.arange(S, dtype=np.int64))
    tri = np.ascontiguousarray(
        (np.arange(128)[:, None] <= np.arange(128)[None, :]).astype(NP_BF16)
    )
    ones = np.ones((128, 128), NP_BF16)

    split_kv = not identity
    nc = _get_program(split_kv)

    in_maps = []
    for c in range(NCORES):
        im = {
            "hT": hT16,
            "wqT": np.ascontiguousarray(wq[c * DQ:(c + 1) * DQ, :].T.astype(NP_BF16)),
            "wkT": np.ascontiguousarray(wk[c * HD:(c + 1) * HD, :].T.astype(NP_BF16)),
            "wvT": np.ascontiguousarray(wv[c * HD:(c + 1) * HD, :].T.astype(NP_BF16)),
            "woT": np.ascontiguousarray(wo[:, c * DQ:(c + 1) * DQ].T.astype(NP_BF16)),
            "coskv": coskv,
            "sinkv": sinkv,
            "trid": tri,
            "onesd": ones,
        }
        if split_kv:
            im["hTkv"] = np.ascontiguousarray(hT16[:, perm])
            cosq, sinq = _rope_tables(position_ids)
            im["cosq"] = cosq
            im["sinq"] = sinq
        in_maps.append(im)

    res = run_bass_kernel_spmd(
        nc, in_maps, core_ids=list(range(NCORES)), **(_run_kwargs or {})
    )
    out = np.zeros((T, HID), np.float32)
    for c in range(NCORES):
        out += res.results[c]["outp"].astype(np.float32)
    kernel.last_results = res  # type: ignore[attr-defined]
    return out


# revision 7
# speedup vs baseline: 1.5571x; 1.1093x over previous
"""Trainium2 Bass kernel for paged-attention Llama-style block (nn_L4maAttention).

Sharding: tensor-parallel over heads across 8 NeuronCores. Core c owns
q-heads [4c, 4c+4), kv-head c, wq/wk/wv row shards and the matching wo
column shard. Each core computes a full [T, HID] partial of the output
projection in bf16; the host sums the 8 partials (the TP reduce).

Device kernel (per core), matmuls in bf16 (full PE rate, half DMA):
  phase 1a: K/V projections in 8 token chunks of 512, PSUM
            double-buffered (psk + psv + transpose bank) x 2 = 6 banks.
            K^T + RoPE -> KT (SBUF resident); V head-dim-major then
            PE-transposed to token-major tiles in V (SBUF resident).
            wq for phase 1b is prefetched here.
  phase 1b: Q projections in 8 chunks of 512, 4 heads x [128,512] PSUM
            (4 banks x 2 = all 8), RoPE -> QT (SBUF resident). RoPE
            half-swap via two partition-shifted SBUF->SBUF DMAs.
  phase 2+3 interleaved per batch b: causal attention for b's 8 (g,it)
            tiles, then b's slice of the output projection, so the
            projection's dense matmul stream overlaps the next batch's
            softmax scalar/vector work. Attention: transposed scores
            [k on partitions, q free] on the causal suffix of each
            128-row k block; exp in bf16 with 1/sqrt(d) folded into the
            activation scale; single [128,128] triangular mask on the
            diagonal sub-block; PV matmuls trail the score matmuls by
            one block so the PE never waits on the exp chain;
            denominators via a ones-matmul (broadcasts over partitions)
            + fast reciprocal.
"""

import math
import sys
from contextlib import ExitStack

import numpy as np

for _p in ("/opt/trn_rl_repo",):
    if _p not in sys.path:
        sys.path.insert(0, _p)

import concourse.mybir as mybir  # noqa: E402
import concourse.tile as tile  # noqa: E402
from concourse import bacc  # noqa: E402
from concourse.bass_utils import run_bass_kernel_spmd  # noqa: E402

NCORES = 8
HID = 4096
NH = 32
NKV = 8
HD = 128
B = 4
S = 1024
T = B * S
GQ = NH // NCORES          # q heads per core = 4
DQ = GQ * HD               # 512
KT32 = HID // 128          # 32 k tiles
QCH = 512                  # token chunk
NQCH = T // QCH            # 8
SCALE = 1.0 / math.sqrt(HD)

FP32 = mybir.dt.float32
BF16 = mybir.dt.bfloat16
NP_BF16 = mybir.dt.np(BF16)

_PROG_CACHE: dict = {}


def _llama31_freqs_np(head_dim: int) -> np.ndarray:
    half = head_dim // 2
    theta, scale, low_ff, high_ff, old_ctx = 500000.0, 8.0, 1.0, 4.0, 8192.0
    freq = 1.0 / (theta ** (np.arange(half, dtype=np.float64) * 2.0 / head_dim))
    wavelen = 2.0 * np.pi / freq
    low_wl, high_wl = old_ctx / low_ff, old_ctx / high_ff
    smooth = (old_ctx / wavelen - low_ff) / (high_ff - low_ff)
    out = np.where(
        wavelen < high_wl,
        freq,
        np.where(wavelen > low_wl, freq / scale, (1.0 - smooth) * freq / scale + smooth * freq),
    )
    return out.astype(np.float64)


def _rope_tables(pos: np.ndarray) -> tuple[np.ndarray, np.ndarray]:
    """cosF [128, n]: cos duplicated on both partition halves.
    sinF2 [128, n]: +sin on rows 0-63, -sin on rows 64-127. The kernel
    computes out = x*cosF + halfswap(x*sinF2), which equals rotate-half
    RoPE."""
    freqs = _llama31_freqs_np(HD)
    ang = pos.astype(np.float64)[None, :] * freqs[:, None]  # [64, n]
    c = np.cos(ang).astype(np.float32)
    s = np.sin(ang).astype(np.float32)
    cosF = np.concatenate([c, c], axis=0)
    sinF2 = np.concatenate([s, -s], axis=0)
    return np.ascontiguousarray(cosF), np.ascontiguousarray(sinF2)


def _build_program(split_kv: bool):
    nc = bacc.Bacc(
        "TRN2",
        target_bir_lowering=False,
        debug=False,
        enable_asserts=False,
        num_devices=NCORES,
    )
    hT = nc.dram_tensor("hT", [HID, T], BF16, kind="ExternalInput")
    hTkv = (
        nc.dram_tensor("hTkv", [HID, T], BF16, kind="ExternalInput") if split_kv else hT
    )
    wqT = nc.dram_tensor("wqT", [HID, DQ], BF16, kind="ExternalInput")
    wkT = nc.dram_tensor("wkT", [HID, HD], BF16, kind="ExternalInput")
    wvT = nc.dram_tensor("wvT", [HID, HD], BF16, kind="ExternalInput")
    woT = nc.dram_tensor("woT", [DQ, HID], BF16, kind="ExternalInput")
    # K (page-rank order) positions are always 0..S-1 per sequence; a
    # [128, S] table sliced modulo S covers both passes in the identity
    # case and the KV pass in the permuted case.
    coskv = nc.dram_tensor("coskv", [128, S], FP32, kind="ExternalInput")
    sinkv = nc.dram_tensor("sinkv", [128, S], FP32, kind="ExternalInput")
    if split_kv:
        cosq = nc.dram_tensor("cosq", [128, T], FP32, kind="ExternalInput")
        sinq = nc.dram_tensor("sinq", [128, T], FP32, kind="ExternalInput")
    trid = nc.dram_tensor("trid", [128, 128], BF16, kind="ExternalInput")
    onesd = nc.dram_tensor("onesd", [128, 128], BF16, kind="ExternalInput")
    identd = nc.dram_tensor("identd", [128, 128], BF16, kind="ExternalInput")
    outp = nc.dram_tensor("outp", [T, HID], BF16, kind="ExternalOutput")

    with tile.TileContext(nc) as tc, ExitStack() as ctx:
        const_pool = ctx.enter_context(tc.tile_pool(name="const", bufs=1))
        QT = const_pool.tile([128, GQ, T], BF16)        # 32KB/part
        KT = const_pool.tile([128, T], BF16)            # 8KB
        V = const_pool.tile([128, T // 128, HD], BF16)  # 8KB (token-major tiles)
        aoT = const_pool.tile([128, GQ, T], BF16)       # 32KB
        tri_sb = const_pool.tile([128, 128], BF16)
        ones_sb = const_pool.tile([128, 128], BF16)
        ident_sb = const_pool.tile([128, 128], BF16)
        coskv_sb = const_pool.tile([128, S], FP32)
        sinkv_sb = const_pool.tile([128, S], FP32)
        nc.sync.dma_start(tri_sb[:], trid.ap()[:, :])
        nc.sync.dma_start(ones_sb[:], onesd.ap()[:, :])
        nc.sync.dma_start(ident_sb[:], identd.ap()[:, :])
        nc.sync.dma_start(coskv_sb[:], coskv.ap()[:, :])
        nc.sync.dma_start(sinkv_sb[:], sinkv.ap()[:, :])
        if split_kv:
            cosq_sb = const_pool.tile([128, T], FP32)
            sinq_sb = const_pool.tile([128, T], FP32)
            nc.sync.dma_start(cosq_sb[:], cosq.ap()[:, :])
            nc.sync.dma_start(sinq_sb[:], sinq.ap()[:, :])
        else:
            cosq_sb, sinq_sb = coskv_sb, sinkv_sb

        def rope_out(ps, cos_a, sin_a, out_full, tpool, n):
            """out = ps*cos + halfswap(ps*sinF2). The half-swap is two
            partition-shifted SBUF->SBUF DMAs (TensorTensor operands must
            share a start partition)."""
            t1 = tpool.tile([128, n], FP32, tag="t1", name="t1")
            u = tpool.tile([128, n], FP32, tag="u", name="u")
            u2 = tpool.tile([128, n], FP32, tag="u2", name="u2")
            nc.vector.tensor_mul(t1[:], ps[:], cos_a)
            nc.vector.tensor_mul(u[:], ps[:], sin_a)
            nc.sync.dma_start(u2[0:64, :], u[64:128, :])
            nc.sync.dma_start(u2[64:128, :], u[0:64, :])
            nc.vector.tensor_add(out_full, t1[:], u2[:])

        # wq prefetch pool spans phases 1a+1b.
        with tc.tile_pool(name="wq", bufs=1) as wqpool:
            wq_sb = wqpool.tile([128, KT32, DQ], BF16)

            # ------------- phase 1a: K/V projections ---------------------
            hkv_r = hTkv.ap().rearrange("(ko p) t -> p ko t", p=128)
            wk_r = wkT.ap().rearrange("(ko p) d -> p ko d", p=128)
            wv_r = wvT.ap().rearrange("(ko p) d -> p ko d", p=128)
            wq_r = wqT.ap().rearrange("(ko p) d -> p ko d", p=128)
            with tc.tile_pool(name="wkv", bufs=1) as wkvpool, \
                 tc.tile_pool(name="h1a", bufs=4) as hpool, \
                 tc.tile_pool(name="ps1a", bufs=2, space="PSUM") as ppool, \
                 tc.tile_pool(name="st1a", bufs=2) as stpool:
                wk_sb = wkvpool.tile([128, KT32, HD], BF16)
                wv_sb = wkvpool.tile([128, KT32, HD], BF16)
                for kg in range(0, KT32, 8):
                    nc.sync.dma_start(wk_sb[:, kg:kg + 8, :], wk_r[:, kg:kg + 8, :])
                    nc.sync.dma_start(wv_sb[:, kg:kg + 8, :], wv_r[:, kg:kg + 8, :])
                for kg in range(0, KT32, 4):
                    nc.sync.dma_start(wq_sb[:, kg:kg + 4, :], wq_r[:, kg:kg + 4, :])
                for c in range(NQCH):
                    tsl = slice(c * QCH, (c + 1) * QCH)
                    hts = []
                    for j in range(4):
                        ht = hpool.tile([128, 8, QCH], BF16, tag="h", name="h")
                        nc.sync.dma_start(ht[:], hkv_r[:, j * 8:(j + 1) * 8, tsl])
                        hts.append(ht)
                    psk = ppool.tile([128, QCH], FP32, tag="psk", name="psk")
                    psv = ppool.tile([128, QCH], FP32, tag="psv", name="psv")
                    for k in range(KT32):
                        ht = hts[k // 8][:, k % 8, :]
                        st = k == 0
                        sp = k == KT32 - 1
                        nc.tensor.matmul(psk[:], wk_sb[:, k, :], ht, start=st, stop=sp)
                        nc.tensor.matmul(psv[:], wv_sb[:, k, :], ht, start=st, stop=sp)
                    p0 = (c * QCH) % S
                    rope_out(
                        psk, coskv_sb[:, p0:p0 + QCH], sinkv_sb[:, p0:p0 + QCH],
                        KT[:, tsl], stpool, QCH,
                    )
                    vsb = stpool.tile([128, QCH], BF16, tag="vsb", name="vsb")
                    nc.scalar.copy(vsb[:], psv[:])
                    pst = ppool.tile([128, 4, 128], BF16, tag="pst", name="pst")
                    for i in range(4):
                        nc.tensor.transpose(
                            pst[:, i, :], vsb[:, i * 128:(i + 1) * 128], ident_sb[:]
                        )
                        nc.vector.tensor_copy(V[:, 4 * c + i, :], pst[:, i, :])

            # ------------- phase 1b: Q projections -----------------------
            h_r = hT.ap().rearrange("(ko p) t -> p ko t", p=128)
            with tc.tile_pool(name="h1b", bufs=6) as hpool, \
                 tc.tile_pool(name="ps1b", bufs=2, space="PSUM") as ppool, \
                 tc.tile_pool(name="st1b", bufs=2) as stpool:
                for c in range(NQCH):
                    tsl = slice(c * QCH, (c + 1) * QCH)
                    hts = []
                    for j in range(4):
                        ht = hpool.tile([128, 8, QCH], BF16, tag="h", name="h")
                        nc.sync.dma_start(ht[:], h_r[:, j * 8:(j + 1) * 8, tsl])
                        hts.append(ht)
                    psq = [
                        ppool.tile([128, QCH], FP32, tag=f"psq{g}", name=f"psq{g}")
                        for g in range(GQ)
                    ]
                    for k in range(KT32):
                        ht = hts[k // 8][:, k % 8, :]
                        st = k == 0
                        sp = k == KT32 - 1
                        for g in range(GQ):
                            nc.tensor.matmul(
                                psq[g][:], wq_sb[:, k, g * 128:(g + 1) * 128], ht,
                                start=st, stop=sp,
                            )
                    if split_kv:
                        cs, ss = cosq_sb[:, tsl], sinq_sb[:, tsl]
                    else:
                        p0 = (c * QCH) % S
                        cs, ss = cosq_sb[:, p0:p0 + QCH], sinq_sb[:, p0:p0 + QCH]
                    for g in range(GQ):
                        rope_out(psq[g], cs, ss, QT[:, g, tsl], stpool, QCH)

        # ------------- phases 2+3, interleaved per batch ------------------
        wo_r = woT.ap().rearrange("(g p) e -> p g e", p=128)
        with tc.tile_pool(name="wo", bufs=1) as wopool:
            wo_sb = wopool.tile([128, GQ, HID], BF16)
            for g in range(GQ):
                nc.sync.dma_start(wo_sb[:, g, :], wo_r[:, g, :])

            with tc.tile_pool(name="sb2", bufs=2) as sbpool, \
                 tc.tile_pool(name="ex2", bufs=10) as expool, \
                 tc.tile_pool(name="pss2", bufs=2, space="PSUM") as pspool, \
                 tc.tile_pool(name="pv2", bufs=2, space="PSUM") as pvpool, \
                 tc.tile_pool(name="psd2", bufs=2, space="PSUM") as pdpool, \
                 tc.tile_pool(name="ps3", bufs=2, space="PSUM") as p3pool, \
                 tc.tile_pool(name="ob3", bufs=3) as obpool:
                for b in range(B):
                    # ---- phase 2 for batch b ----
                    for g in range(GQ):
                        for it in range(2):
                            qoff = it * QCH
                            q0 = b * S + qoff
                            njt = (qoff + QCH) // 128
                            es = sbpool.tile([128, QCH], BF16, tag="es", name="es")
                            pv = pvpool.tile([128, QCH], FP32, tag="pv", name="pv")
                            exs, offs = [], []
                            # Score matmuls stream ahead; PV matmuls trail
                            # by one so the PE never waits on exp.
                            for jt in range(njt):
                                ko = b * S + jt * 128
                                off = jt * 128 - qoff if jt * 128 >= qoff else 0
                                pss = pspool.tile(
                                    [128, QCH], FP32, tag="pss", name="pss"
                                )
                                nc.tensor.matmul(
                                    pss[:, off:QCH],
                                    KT[:, ko:ko + 128],
                                    QT[:, g, q0 + off:q0 + QCH],
                                    start=True, stop=True,
                                )
                                ex = expool.tile([128, QCH], BF16, tag="ex", name="ex")
                                nc.scalar.activation(
                                    ex[:, off:QCH], pss[:, off:QCH],
                                    mybir.ActivationFunctionType.Exp, scale=SCALE,
                                )
                                if jt * 128 >= qoff:  # diagonal block
                                    nc.vector.tensor_mul(
                                        ex[:, off:off + 128], ex[:, off:off + 128],
                                        tri_sb[:],
                                    )
                                if jt == 0:
                                    nc.vector.tensor_copy(es[:], ex[:])
                                else:
                                    nc.vector.tensor_add(
                                        es[:, off:QCH], es[:, off:QCH], ex[:, off:QCH]
                                    )
                                exs.append(ex)
                                offs.append(off)
                                if jt >= 1:
                                    pj, po = jt - 1, offs[jt - 1]
                                    nc.tensor.matmul(
                                        pv[:, po:QCH],
                                        V[:, b * 8 + pj, :],
                                        exs[pj][:, po:QCH],
                                        start=(pj == 0), stop=False,
                                    )
                            pj, po = njt - 1, offs[njt - 1]
                            nc.tensor.matmul(
                                pv[:, po:QCH],
                                V[:, b * 8 + pj, :],
                                exs[pj][:, po:QCH],
                                start=(pj == 0), stop=True,
                            )
                            psd = pdpool.tile([128, QCH], FP32, tag="psd", name="psd")
                            nc.tensor.matmul(
                                psd[:], ones_sb[:], es[:], start=True, stop=True
                            )
                            rec = sbpool.tile([128, QCH], FP32, tag="rec", name="rec")
                            nc.vector.reciprocal_approx_fast(rec[:], psd[:])
                            nc.vector.tensor_mul(
                                aoT[:, g, q0:q0 + QCH], pv[:], rec[:]
                            )
                    # ---- phase 3 for batch b's tokens ----
                    for eg in range(2):
                        for tb in range(8 * b, 8 * b + 8):
                            ob = obpool.tile([128, 4, QCH], BF16, tag="ob", name="ob")
                            for ei in range(4):
                                e0 = eg * 2048 + ei * QCH
                                pso = p3pool.tile(
                                    [128, QCH], FP32, tag="pso", name="pso"
                                )
                                for g in range(GQ):
                                    nc.tensor.matmul(
                                        pso[:],
                                        aoT[:, g, tb * 128:(tb + 1) * 128],
                                        wo_sb[:, g, e0:e0 + QCH],
                                        start=(g == 0), stop=(g == GQ - 1),
                                    )
                                if ei % 2 == 0:
                                    nc.scalar.copy(ob[:, ei, :], pso[:])
                                else:
                                    nc.vector.tensor_copy(ob[:, ei, :], pso[:])
                            nc.sync.dma_start(
                                outp.ap()[tb * 128:(tb + 1) * 128,
                                          eg * 2048:(eg + 1) * 2048],
                                ob[:],
                            )

    nc.finalize()
    return nc


def _get_program(split_kv: bool):
    if split_kv not in _PROG_CACHE:
        _PROG_CACHE[split_kv] = _build_program(split_kv)
    return _PROG_CACHE[split_kv]


def kernel(
    hidden_states, wq, wk, wv, wo, kv_cache, position_ids,
    kv_page_indices, kv_page_indptr, kv_last_page_lens, qo_indptr,
    _run_kwargs: dict | None = None,
):
    hidden_states = np.asarray(hidden_states, np.float32)
    wq = np.asarray(wq, np.float32)
    wk = np.asarray(wk, np.float32)
    wv = np.asarray(wv, np.float32)
    wo = np.asarray(wo, np.float32)
    position_ids = np.asarray(position_ids, np.int32)
    qo_indptr = np.asarray(qo_indptr, np.int64)

    nnz = hidden_states.shape[0]
    b = qo_indptr.shape[0] - 1
    assert nnz == T and b == B, (nnz, b)
    assert np.array_equal(qo_indptr, np.arange(B + 1, dtype=np.int64) * S), (
        "kernel assumes uniform sequence lengths of 1024"
    )

    # Page-gather order: the reference gathers pages in list order, so the
    # token with position p within its sequence lands at page-order rank p.
    # KV must be fed in rank order; the q path stays in token order.
    perm = np.empty(T, np.int64)
    identity = True
    for bi in range(B):
        pos_b = position_ids[bi * S:(bi + 1) * S].astype(np.int64)
        assert np.array_equal(np.sort(pos_b), np.arange(S)), (
            "kernel assumes positions cover 0..S-1 exactly once per sequence"
        )
        inv = np.empty(S, np.int64)
        inv[pos_b] = np.arange(S)
        perm[bi * S:(bi + 1) * S] = bi * S + inv
        if not np.array_equal(inv, np.arange(S)):
            identity = False

    hT16 = np.ascontiguousarray(hidden_states.T.astype(NP_BF16))
    coskv, sinkv = _rope_tables(np.arange(S, dtype=np.int64))
    tri = np.ascontiguousarray(
        (np.arange(128)[:, None] <= np.arange(128)[None, :]).astype(NP_BF16)
    )
    ones = np.ones((128, 128), NP_BF16)
    eye = np.eye(128, dtype=np.float32).astype(NP_BF16)

    split_kv = not identity
    nc = _get_program(split_kv)

    in_maps = []
    for c in range(NCORES):
        im = {
            "hT": hT16,
            "wqT": np.ascontiguousarray(wq[c * DQ:(c + 1) * DQ, :].T.astype(NP_BF16)),
            "wkT": np.ascontiguousarray(wk[c * HD:(c + 1) * HD, :].T.astype(NP_BF16)),
            "wvT": np.ascontiguousarray(wv[c * HD:(c + 1) * HD, :].T.astype(NP_BF16)),
            "woT": np.ascontiguousarray(wo[:, c * DQ:(c + 1) * DQ].T.astype(NP_BF16)),
            "coskv": coskv,
            "sinkv": sinkv,
            "trid": tri,
            "onesd": ones,
            "identd": eye,
        }
        if split_kv:
            im["hTkv"] = np.ascontiguousarray(hT16[:, perm])
            cosq, sinq = _rope_tables(position_ids)
            im["cosq"] = cosq
            im["sinq"] = sinq
        in_maps.append(im)

    res = run_bass_kernel_spmd(
        nc, in_maps, core_ids=list(range(NCORES)), **(_run_kwargs or {})
    )
    out = np.zeros((T, HID), np.float32)
    for c in range(NCORES):
        out += res.results[c]["outp"].astype(np.float32)
    kernel.last_results = res  # type: ignore[attr-defined]
    return out
